# revision 15
# baseline (speedup 1.0000x reference)
"""Distance-based cross-entropy loss (DCE) on 8 TRN2 NeuronCores.

reference math:
    d[c,k]  = ||prototypes[c,k,:] - feature||^2          (C=10000, K=4, D=2048)
    logits  = -GAMMA * d
    log_one = logsumexp(logits)   (over all C*K)
    out     = sum_k (log_one - logits[label, k])

Sharding: classes split evenly across 8 cores (1250 classes = 5000 rows of
2048 each).  Each core streams its ~41 MB shard once (memory bound): DVE
subtracts the broadcast feature, ACT squares + row-reduces (accum_out) into
per-row distances d, then a per-partition min (DVE) and exp(m_p - d) row-sum
(ACT) produce 128 logsumexp partials per core.  The 8*128 partials plus the
raw d values are gathered; the scalar denominator "all-reduce" and the
4-element numerator lookup happen on host in float64.
"""

import numpy as np

import concourse.bacc as bacc
import concourse.bass as bass
import concourse.mybir as mybir
import concourse.tile as tile
from concourse.bass_utils import run_bass_kernel_spmd

GAMMA = 1.0
C, K, D = 10000, 4, 2048
N_CORES = 8
CPC = C // N_CORES          # classes per core
R = CPC * K                 # rows per core = 5000
A = 5                       # row-groups of 128 per DMA tile
TILE_ROWS = 128 * A         # 640 rows per DMA tile
NT = 8                      # DMA tiles per core (last one partial: 520 rows)
NCOLS = NT * A              # d columns per partition = 40
FILL = 3.0e38               # unused d_sb slots -> exp underflows to 0
TAIL_ROWS = 8               # R = 39*128 + 8 ragged rows

# (start_row, full 128-row groups, carries the 8-row tail) — the ragged tile
# first, then big tiles, shrinking at the end to minimize the serial tail
TILES = (
    [(4480, 4, True)]
    + [(i * 640, 5, False) for i in range(6)]
    + [(3840, 2, False), (4096, 2, False), (4352, 1, False)]
)

_f32 = mybir.dt.float32


def _build_bass():
    nc = bacc.Bacc("TRN2")
    p_h = nc.dram_tensor("p", [R, D], _f32, kind="ExternalInput")
    f_h = nc.dram_tensor("f", [D], _f32, kind="ExternalInput")
    # one output: cols 0..39 = d, col 40 = row_min, col 41 = s_row
    out_a = nc.dram_tensor("out_a", [128, NCOLS + 2], _f32, kind="ExternalOutput")

    with tile.TileContext(nc) as tc:
        with (
            tc.tile_pool(name="work", bufs=4) as work,
            tc.tile_pool(name="singles", bufs=1) as singles,
        ):
            # feature broadcast to all 128 partitions (partition-step-0 DMA)
            f_bcast = singles.tile([128, D], _f32)
            f_ap = f_h[:]
            f_b_ap = bass.AP(
                tensor=f_ap.tensor,
                offset=f_ap.offset,
                ap=[[0, 128]] + list(f_ap.ap),
            )
            nc.gpsimd.dma_start(out=f_bcast[:, :], in_=f_b_ap)

            # unused d_sb entries (last tile, col 39, partitions 8..127) must
            # read as +huge so they lose the min and underflow the exp
            d_sb = singles.tile([128, NCOLS], _f32)
            nc.gpsimd.memset(d_sb[:, :], FILL)

            # row r -> d_sb[r % 128, r // 128].  Tile sizes shrink toward the
            # end of the stream so the kernel's serial tail (compute of the
            # last-loaded tile) is a single slice, and the ragged 8-row tail
            # rides in the first tile.
            for start, ng, has_tail in TILES:
                p_tile = work.tile([128, A, D], _f32)
                if ng:
                    view = p_h[start : start + ng * 128, :].rearrange(
                        "(a q) d -> q a d", q=128
                    )
                    nc.sync.dma_start(out=p_tile[:, 0:ng, :], in_=view)
                if has_tail:
                    nc.sync.dma_start(
                        out=p_tile[0:TAIL_ROWS, ng, :],
                        in_=p_h[R - TAIL_ROWS : R, :],
                    )
                for a in range(ng + (1 if has_tail else 0)):
                    np_ = 128 if a < ng else TAIL_ROWS
                    col = start // 128 + a
                    sl = p_tile[0:np_, a, :]
                    nc.vector.tensor_sub(sl, sl, f_bcast[0:np_, :])
                    nc.scalar.activation(
                        out=sl,
                        in_=sl,
                        func=mybir.ActivationFunctionType.Square,
                        accum_out=d_sb[0:np_, col : col + 1],
                    )

            # per-partition logsumexp partials
            row_min = singles.tile([128, 1], _f32)
            nc.vector.tensor_reduce(
                out=row_min[:, :],
                in_=d_sb[:, :],
                axis=mybir.AxisListType.X,
                op=mybir.AluOpType.min,
            )
            e_sb = singles.tile([128, NCOLS], _f32)
            s_row = singles.tile([128, 1], _f32)
            nc.scalar.activation(
                out=e_sb[:, :],
                in_=d_sb[:, :],
                func=mybir.ActivationFunctionType.Exp,
                bias=row_min[:, :],
                scale=-GAMMA,
                accum_out=s_row[:, :],
            )

            nc.sync.dma_start(out=out_d[:, :], in_=d_sb[:, :])
            nc.sync.dma_start(out=out_m[:, :], in_=row_min[:, :])
            nc.sync.dma_start(out=out_s[:, :], in_=s_row[:, :])

    nc.compile()
    return nc


def run(feature, label, all_prototypes, trace=False):
    """Returns (output_scalar, BassKernelResults)."""
    feature = np.ascontiguousarray(np.asarray(feature), dtype=np.float32)
    P = np.asarray(all_prototypes, dtype=np.float32).reshape(C * K, D)
    lbl = int(label)

    nc = _build_bass()
    in_maps = []
    for c in range(N_CORES):
        shard = np.ascontiguousarray(P[c * R : (c + 1) * R])
        in_maps.append({"p": shard, "f": feature})

    res = run_bass_kernel_spmd(
        nc, in_maps, core_ids=list(range(N_CORES)), trace=trace
    )
    outs = res.results

    m = np.stack([o["out_m"][:, 0] for o in outs]).astype(np.float64)  # [8,128]
    s = np.stack([o["out_s"][:, 0] for o in outs]).astype(np.float64)  # [8,128]
    dsb = [o["out_d"] for o in outs]  # each [128, NCOLS] f32

    # all-reduce the scalar denominator (in log space, f64)
    M = float(m.min())
    one = float((s * np.exp(GAMMA * (M - m))).sum())
    log_one = np.log(one) - GAMMA * M

    # numerator: the K rows of the label class live on one shard
    owner, lc = divmod(lbl, CPC)
    dsum = 0.0
    for k in range(K):
        r = lc * K + k
        dsum += float(dsb[owner][r % 128, r // 128])

    prob = K * log_one + GAMMA * dsum
    return np.float32(prob), res


def kernel(feature, label, all_prototypes):
    out, _ = run(feature, label, all_prototypes)
    return out


# revision 22
# speedup vs baseline: 1.0702x; 1.0702x over previous
"""Distance-based cross-entropy loss (DCE) on 8 TRN2 NeuronCores.

reference math:
    d[c,k]  = ||prototypes[c,k,:] - feature||^2          (C=10000, K=4, D=2048)
    logits  = -GAMMA * d
    log_one = logsumexp(logits)   (over all C*K)
    out     = sum_k (log_one - logits[label, k])

Sharding: classes split evenly across 8 cores (1250 classes = 5000 rows of
2048 each).  Each core streams its ~41 MB shard once (memory bound): DVE
subtracts the broadcast feature, ACT squares + row-reduces (accum_out) into
per-row distances d, then a per-partition min (DVE) and exp(m_p - d) row-sum
(ACT) produce 128 logsumexp partials per core.  The 8*128 partials plus the
raw d values are gathered; the scalar denominator "all-reduce" and the
4-element numerator lookup happen on host in float64.
"""

import numpy as np

import concourse.bacc as bacc
import concourse.bass as bass
import concourse.mybir as mybir
import concourse.tile as tile
from concourse.bass_utils import run_bass_kernel_spmd

GAMMA = 1.0
C, K, D = 10000, 4, 2048
N_CORES = 8
CPC = C // N_CORES          # classes per core
R = CPC * K                 # rows per core = 5000
A = 5                       # row-groups of 128 per DMA tile
TILE_ROWS = 128 * A         # 640 rows per DMA tile
NT = 8                      # DMA tiles per core (last one partial: 520 rows)
NCOLS = NT * A              # d columns per partition = 40
FILL = 3.0e38               # unused d_sb slots -> exp underflows to 0
TAIL_ROWS = 8               # R = 39*128 + 8 ragged rows

# (start_row, full 128-row groups, carries the 8-row tail) — the ragged tile
# first, then big tiles, tapering at the end so the DVE backlog drains and
# the kernel's serial tail is a single slice
TILES = (
    [(4480, 4, True)]
    + [(i * 640, 5, False) for i in range(5)]
    + [(3200, 4, False), (3712, 3, False), (4096, 2, False), (4352, 1, False)]
)

_f32 = mybir.dt.float32


def _build_bass():
    nc = bacc.Bacc("TRN2")
    p_h = nc.dram_tensor("p", [R, D], _f32, kind="ExternalInput")
    f_h = nc.dram_tensor("f", [D], _f32, kind="ExternalInput")
    # one output: cols 0..39 = d, col 40 = row_min, col 41 = s_row
    out_a = nc.dram_tensor("out_a", [128, NCOLS + 2], _f32, kind="ExternalOutput")

    with tile.TileContext(nc) as tc:
        with (
            tc.tile_pool(name="work", bufs=4) as work,
            tc.tile_pool(name="singles", bufs=1) as singles,
            tc.tile_pool(name="psum", bufs=1, space="PSUM") as psum_pool,
        ):
            # broadcast f to 128 partitions via PE (ones ⊗ f) — an 8 KB DMA
            # plus idle-TensorE work instead of a 1 MB broadcast DMA
            f_ap = f_h[:]
            f_sb = singles.tile([1, D], _f32)
            nc.sync.dma_start(
                out=f_sb[0:1, :],
                in_=bass.AP(
                    tensor=f_ap.tensor,
                    offset=f_ap.offset,
                    ap=[[0, 1]] + list(f_ap.ap),
                ),
            )
            ones = singles.tile([1, 128], _f32)
            nc.vector.memset(ones[:, :], 1.0)
            psum_fb = psum_pool.tile([128, D], _f32)
            for j in range(D // 512):
                nc.tensor.matmul(
                    psum_fb[:, j * 512 : (j + 1) * 512],
                    ones[0:1, :],
                    f_sb[0:1, j * 512 : (j + 1) * 512],
                    start=True,
                    stop=True,
                )
            f_bcast = singles.tile([128, D], _f32)
            nc.vector.tensor_copy(out=f_bcast[:, :], in_=psum_fb[:, :])

            # all results live in one tile: cols 0..39 = d, 40 = min, 41 = s.
            # Unused d entries (col 39, partitions 8..127) must read as +huge
            # so they lose the min and underflow the exp.
            d_all = singles.tile([128, NCOLS + 2], _f32)
            d_sb = d_all[:, 0:NCOLS]
            nc.gpsimd.memset(d_all[:, :], FILL)

            # row r -> d_sb[r % 128, r // 128].  Tile sizes shrink toward the
            # end of the stream so the kernel's serial tail (compute of the
            # last-loaded tile) is a single slice, and the ragged 8-row tail
            # rides in the first tile.
            for start, ng, has_tail in TILES:
                p_tile = work.tile([128, A, D], _f32)
                if ng:
                    view = p_h[start : start + ng * 128, :].rearrange(
                        "(a q) d -> q a d", q=128
                    )
                    nc.sync.dma_start(out=p_tile[:, 0:ng, :], in_=view)
                if has_tail:
                    nc.sync.dma_start(
                        out=p_tile[0:TAIL_ROWS, ng, :],
                        in_=p_h[R - TAIL_ROWS : R, :],
                    )
                for a in range(ng + (1 if has_tail else 0)):
                    np_ = 128 if a < ng else TAIL_ROWS
                    col = start // 128 + a
                    sl = p_tile[0:np_, a, :]
                    nc.vector.tensor_sub(sl, sl, f_bcast[0:np_, :])
                    nc.scalar.activation(
                        out=sl,
                        in_=sl,
                        func=mybir.ActivationFunctionType.Square,
                        accum_out=d_sb[0:np_, col : col + 1],
                    )

            # per-partition logsumexp partials
            row_min = d_all[:, NCOLS : NCOLS + 1]
            nc.vector.tensor_reduce(
                out=row_min,
                in_=d_sb,
                axis=mybir.AxisListType.X,
                op=mybir.AluOpType.min,
            )
            e_sb = singles.tile([128, NCOLS], _f32)
            s_row = d_all[:, NCOLS + 1 : NCOLS + 2]
            nc.scalar.activation(
                out=e_sb[:, :],
                in_=d_sb,
                func=mybir.ActivationFunctionType.Exp,
                bias=row_min,
                scale=-GAMMA,
                accum_out=s_row,
            )

            nc.sync.dma_start(out=out_a[:, :], in_=d_all[:, :])

    nc.compile()
    return nc


def run(feature, label, all_prototypes, trace=False):
    """Returns (output_scalar, BassKernelResults)."""
    feature = np.ascontiguousarray(np.asarray(feature), dtype=np.float32)
    P = np.asarray(all_prototypes, dtype=np.float32).reshape(C * K, D)
    lbl = int(label)

    nc = _build_bass()
    in_maps = []
    for c in range(N_CORES):
        shard = np.ascontiguousarray(P[c * R : (c + 1) * R])
        in_maps.append({"p": shard, "f": feature})

    res = run_bass_kernel_spmd(
        nc, in_maps, core_ids=list(range(N_CORES)), trace=trace
    )
    outs = res.results

    m = np.stack([o["out_a"][:, NCOLS] for o in outs]).astype(np.float64)
    s = np.stack([o["out_a"][:, NCOLS + 1] for o in outs]).astype(np.float64)
    dsb = [o["out_a"][:, 0:NCOLS] for o in outs]  # each [128, NCOLS] f32

    # all-reduce the scalar denominator (in log space, f64)
    M = float(m.min())
    one = float((s * np.exp(GAMMA * (M - m))).sum())
    log_one = np.log(one) - GAMMA * M

    # numerator: the K rows of the label class live on one shard
    owner, lc = divmod(lbl, CPC)
    dsum = 0.0
    for k in range(K):
        r = lc * K + k
        dsum += float(dsb[owner][r % 128, r // 128])

    prob = K * log_one + GAMMA * dsum
    return np.float32(prob), res


def kernel(feature, label, all_prototypes):
    out, _ = run(feature, label, all_prototypes)
    return out


# revision 25
# speedup vs baseline: 1.0827x; 1.0117x over previous
"""Distance-based cross-entropy loss (DCE) on 8 TRN2 NeuronCores.

reference math:
    d[c,k]  = ||prototypes[c,k,:] - feature||^2          (C=10000, K=4, D=2048)
    logits  = -GAMMA * d
    log_one = logsumexp(logits)   (over all C*K)
    out     = sum_k (log_one - logits[label, k])

Sharding: classes split evenly across 8 cores (1250 classes = 5000 rows of
2048 each).  Each core streams its ~41 MB shard once (memory bound): DVE
subtracts the broadcast feature, ACT squares + row-reduces (accum_out) into
per-row distances d, then a per-partition min (DVE) and exp(m_p - d) row-sum
(ACT) produce 128 logsumexp partials per core.  The 8*128 partials plus the
raw d values are gathered; the scalar denominator "all-reduce" and the
4-element numerator lookup happen on host in float64.
"""

import numpy as np

import concourse.bacc as bacc
import concourse.bass as bass
import concourse.mybir as mybir
import concourse.tile as tile
from concourse.bass_utils import run_bass_kernel_spmd

GAMMA = 1.0
C, K, D = 10000, 4, 2048
N_CORES = 8
CPC = C // N_CORES          # classes per core
R = CPC * K                 # rows per core = 5000
A = 5                       # row-groups of 128 per DMA tile
TILE_ROWS = 128 * A         # 640 rows per DMA tile
NT = 8                      # DMA tiles per core (last one partial: 520 rows)
NCOLS = NT * A              # d columns per partition = 40
FILL = 3.0e38               # unused d_sb slots -> exp underflows to 0
TAIL_ROWS = 8               # R = 39*128 + 8 ragged rows

# (start_row, full 128-row groups, carries the 8-row tail) — the ragged tile
# first, then big tiles, tapering at the end so the DVE backlog drains and
# the kernel's serial tail is a single slice
TILES = (
    [(4480, 4, True)]
    + [(i * 640, 5, False) for i in range(4)]
    + [(2560, 4, False), (3072, 4, False), (3584, 3, False),
       (3968, 2, False), (4224, 1, False), (4352, 1, False)]
)

_f32 = mybir.dt.float32


def _build_bass():
    nc = bacc.Bacc("TRN2")
    p_h = nc.dram_tensor("p", [R, D], _f32, kind="ExternalInput")
    f_h = nc.dram_tensor("f", [D], _f32, kind="ExternalInput")
    # one output: cols 0..39 = d, col 40 = row_min, col 41 = s_row
    out_a = nc.dram_tensor("out_a", [128, NCOLS + 2], _f32, kind="ExternalOutput")

    with tile.TileContext(nc) as tc:
        with (
            tc.tile_pool(name="work", bufs=4) as work,
            tc.tile_pool(name="singles", bufs=1) as singles,
            tc.tile_pool(name="psum", bufs=1, space="PSUM") as psum_pool,
        ):
            # broadcast f to 128 partitions via PE (ones ⊗ f) — an 8 KB DMA
            # plus idle-TensorE work instead of a 1 MB broadcast DMA
            f_ap = f_h[:]
            f_sb = singles.tile([1, D], _f32)
            nc.sync.dma_start(
                out=f_sb[0:1, :],
                in_=bass.AP(
                    tensor=f_ap.tensor,
                    offset=f_ap.offset,
                    ap=[[0, 1]] + list(f_ap.ap),
                ),
            )
            ones = singles.tile([1, 128], _f32)
            nc.vector.memset(ones[:, :], 1.0)
            psum_fb = psum_pool.tile([128, D], _f32)
            for j in range(D // 512):
                nc.tensor.matmul(
                    psum_fb[:, j * 512 : (j + 1) * 512],
                    ones[0:1, :],
                    f_sb[0:1, j * 512 : (j + 1) * 512],
                    start=True,
                    stop=True,
                )
            f_bcast = singles.tile([128, D], _f32)
            nc.vector.tensor_copy(out=f_bcast[:, :], in_=psum_fb[:, :])

            # all results live in one tile: cols 0..39 = d, 40 = min, 41 = s.
            # Unused d entries (col 39, partitions 8..127) must read as +huge
            # so they lose the min and underflow the exp.
            d_all = singles.tile([128, NCOLS + 2], _f32)
            d_sb = d_all[:, 0:NCOLS]
            nc.gpsimd.memset(d_all[:, :], FILL)

            # row r -> d_sb[r % 128, r // 128].  Tile sizes shrink toward the
            # end of the stream so the kernel's serial tail (compute of the
            # last-loaded tile) is a single slice, and the ragged 8-row tail
            # rides in the first tile.
            for start, ng, has_tail in TILES:
                p_tile = work.tile([128, A, D], _f32)
                # split big loads in two so groups unlock at finer grain
                # (the consumer semaphore fires per dma_start)
                chunks = [(0, ng)] if ng <= 2 else [(0, ng - ng // 2), (ng - ng // 2, ng // 2)]
                for c0, cn in chunks:
                    if not cn:
                        continue
                    view = p_h[start + c0 * 128 : start + (c0 + cn) * 128, :].rearrange(
                        "(a q) d -> q a d", q=128
                    )
                    nc.sync.dma_start(out=p_tile[:, c0 : c0 + cn, :], in_=view)
                if has_tail:
                    nc.sync.dma_start(
                        out=p_tile[0:TAIL_ROWS, ng, :],
                        in_=p_h[R - TAIL_ROWS : R, :],
                    )
                for a in range(ng + (1 if has_tail else 0)):
                    np_ = 128 if a < ng else TAIL_ROWS
                    col = start // 128 + a
                    sl = p_tile[0:np_, a, :]
                    nc.vector.tensor_sub(sl, sl, f_bcast[0:np_, :])
                    nc.scalar.activation(
                        out=sl,
                        in_=sl,
                        func=mybir.ActivationFunctionType.Square,
                        accum_out=d_sb[0:np_, col : col + 1],
                    )

            # per-partition logsumexp partials
            row_min = d_all[:, NCOLS : NCOLS + 1]
            nc.vector.tensor_reduce(
                out=row_min,
                in_=d_sb,
                axis=mybir.AxisListType.X,
                op=mybir.AluOpType.min,
            )
            e_sb = singles.tile([128, NCOLS], _f32)
            s_row = d_all[:, NCOLS + 1 : NCOLS + 2]
            nc.scalar.activation(
                out=e_sb[:, :],
                in_=d_sb,
                func=mybir.ActivationFunctionType.Exp,
                bias=row_min,
                scale=-GAMMA,
                accum_out=s_row,
            )

            # ACT-issued HWDGE: the output DMA launches straight from the
            # engine that produced the last result, no cross-engine sem hop
            nc.scalar.dma_start(out=out_a[:, :], in_=d_all[:, :])

    nc.compile()
    return nc


def run(feature, label, all_prototypes, trace=False):
    """Returns (output_scalar, BassKernelResults)."""
    feature = np.ascontiguousarray(np.asarray(feature), dtype=np.float32)
    P = np.asarray(all_prototypes, dtype=np.float32).reshape(C * K, D)
    lbl = int(label)

    nc = _build_bass()
    in_maps = []
    for c in range(N_CORES):
        shard = np.ascontiguousarray(P[c * R : (c + 1) * R])
        in_maps.append({"p": shard, "f": feature})

    res = run_bass_kernel_spmd(
        nc, in_maps, core_ids=list(range(N_CORES)), trace=trace
    )
    outs = res.results

    m = np.stack([o["out_a"][:, NCOLS] for o in outs]).astype(np.float64)
    s = np.stack([o["out_a"][:, NCOLS + 1] for o in outs]).astype(np.float64)
    dsb = [o["out_a"][:, 0:NCOLS] for o in outs]  # each [128, NCOLS] f32

    # all-reduce the scalar denominator (in log space, f64)
    M = float(m.min())
    one = float((s * np.exp(GAMMA * (M - m))).sum())
    log_one = np.log(one) - GAMMA * M

    # numerator: the K rows of the label class live on one shard
    owner, lc = divmod(lbl, CPC)
    dsum = 0.0
    for k in range(K):
        r = lc * K + k
        dsum += float(dsb[owner][r % 128, r // 128])

    prob = K * log_one + GAMMA * dsum
    return np.float32(prob), res


def kernel(feature, label, all_prototypes):
    out, _ = run(feature, label, all_prototypes)
    return out


# revision 26
# speedup vs baseline: 1.0884x; 1.0053x over previous
"""Distance-based cross-entropy loss (DCE) on 8 TRN2 NeuronCores.

reference math:
    d[c,k]  = ||prototypes[c,k,:] - feature||^2          (C=10000, K=4, D=2048)
    logits  = -GAMMA * d
    log_one = logsumexp(logits)   (over all C*K)
    out     = sum_k (log_one - logits[label, k])

Sharding: classes split evenly across 8 cores (1250 classes = 5000 rows of
2048 each).  Each core streams its ~41 MB shard once (memory bound): DVE
subtracts the broadcast feature, ACT squares + row-reduces (accum_out) into
per-row distances d, then a per-partition min (DVE) and exp(m_p - d) row-sum
(ACT) produce 128 logsumexp partials per core.  The 8*128 partials plus the
raw d values are gathered; the scalar denominator "all-reduce" and the
4-element numerator lookup happen on host in float64.
"""

import numpy as np

import concourse.bacc as bacc
import concourse.bass as bass
import concourse.mybir as mybir
import concourse.tile as tile
from concourse.bass_utils import run_bass_kernel_spmd

GAMMA = 1.0
C, K, D = 10000, 4, 2048
N_CORES = 8
CPC = C // N_CORES          # classes per core
R = CPC * K                 # rows per core = 5000
A = 4                       # row-groups of 128 per DMA tile buffer
NCOLS = 40                  # d columns per partition (39 full groups + tail)
FILL = 3.0e38               # unused d_sb slots -> exp underflows to 0
TAIL_ROWS = 8               # R = 39*128 + 8 ragged rows

# (start_row, full 128-row groups, carries the 8-row tail) — the ragged tile
# first, then big tiles, tapering at the end so the DVE backlog drains and
# the kernel's serial tail is a single slice
TILES = (
    [(4608, 3, True)]
    + [(i * 512, 4, False) for i in range(7)]
    + [(3584, 3, False), (3968, 2, False), (4224, 1, False),
       (4352, 1, False), (4480, 1, False)]
)

_f32 = mybir.dt.float32


def _build_bass():
    nc = bacc.Bacc("TRN2")
    p_h = nc.dram_tensor("p", [R, D], _f32, kind="ExternalInput")
    f_h = nc.dram_tensor("f", [D], _f32, kind="ExternalInput")
    # one output: cols 0..39 = d, col 40 = row_min, col 41 = s_row
    out_a = nc.dram_tensor("out_a", [128, NCOLS + 2], _f32, kind="ExternalOutput")

    with tile.TileContext(nc) as tc:
        with (
            tc.tile_pool(name="work", bufs=4) as work,
            tc.tile_pool(name="singles", bufs=1) as singles,
            tc.tile_pool(name="psum", bufs=1, space="PSUM") as psum_pool,
        ):
            # broadcast f to 128 partitions via PE (ones ⊗ f) — an 8 KB DMA
            # plus idle-TensorE work instead of a 1 MB broadcast DMA
            f_ap = f_h[:]
            f_sb = singles.tile([1, D], _f32)
            nc.sync.dma_start(
                out=f_sb[0:1, :],
                in_=bass.AP(
                    tensor=f_ap.tensor,
                    offset=f_ap.offset,
                    ap=[[0, 1]] + list(f_ap.ap),
                ),
            )
            ones = singles.tile([1, 128], _f32)
            nc.vector.memset(ones[:, :], 1.0)
            psum_fb = psum_pool.tile([128, D], _f32)
            for j in range(D // 512):
                nc.tensor.matmul(
                    psum_fb[:, j * 512 : (j + 1) * 512],
                    ones[0:1, :],
                    f_sb[0:1, j * 512 : (j + 1) * 512],
                    start=True,
                    stop=True,
                )
            f_bcast = singles.tile([128, D], _f32)
            nc.vector.tensor_copy(out=f_bcast[:, :], in_=psum_fb[:, :])

            # all results live in one tile: cols 0..39 = d, 40 = min, 41 = s.
            # Unused d entries (col 39, partitions 8..127) must read as +huge
            # so they lose the min and underflow the exp.
            d_all = singles.tile([128, NCOLS + 2], _f32)
            d_sb = d_all[:, 0:NCOLS]
            nc.gpsimd.memset(d_all[:, :], FILL)

            # row r -> d_sb[r % 128, r // 128].  Tile sizes shrink toward the
            # end of the stream so the kernel's serial tail (compute of the
            # last-loaded tile) is a single slice, and the ragged 8-row tail
            # rides in the first tile.
            for start, ng, has_tail in TILES:
                p_tile = work.tile([128, A, D], _f32)
                # split big loads in two so groups unlock at finer grain
                # (the consumer semaphore fires per dma_start)
                chunks = [(0, ng)] if ng <= 2 else [(0, ng - ng // 2), (ng - ng // 2, ng // 2)]
                for c0, cn in chunks:
                    if not cn:
                        continue
                    view = p_h[start + c0 * 128 : start + (c0 + cn) * 128, :].rearrange(
                        "(a q) d -> q a d", q=128
                    )
                    nc.sync.dma_start(out=p_tile[:, c0 : c0 + cn, :], in_=view)
                if has_tail:
                    nc.sync.dma_start(
                        out=p_tile[0:TAIL_ROWS, ng, :],
                        in_=p_h[R - TAIL_ROWS : R, :],
                    )
                for a in range(ng + (1 if has_tail else 0)):
                    np_ = 128 if a < ng else TAIL_ROWS
                    col = start // 128 + a
                    sl = p_tile[0:np_, a, :]
                    nc.vector.tensor_sub(sl, sl, f_bcast[0:np_, :])
                    nc.scalar.activation(
                        out=sl,
                        in_=sl,
                        func=mybir.ActivationFunctionType.Square,
                        accum_out=d_sb[0:np_, col : col + 1],
                    )

            # per-partition logsumexp partials
            row_min = d_all[:, NCOLS : NCOLS + 1]
            nc.vector.tensor_reduce(
                out=row_min,
                in_=d_sb,
                axis=mybir.AxisListType.X,
                op=mybir.AluOpType.min,
            )
            e_sb = singles.tile([128, NCOLS], _f32)
            s_row = d_all[:, NCOLS + 1 : NCOLS + 2]
            nc.scalar.activation(
                out=e_sb[:, :],
                in_=d_sb,
                func=mybir.ActivationFunctionType.Exp,
                bias=row_min,
                scale=-GAMMA,
                accum_out=s_row,
            )

            # ACT-issued HWDGE: the output DMA launches straight from the
            # engine that produced the last result, no cross-engine sem hop
            nc.scalar.dma_start(out=out_a[:, :], in_=d_all[:, :])

    nc.compile()
    return nc


def run(feature, label, all_prototypes, trace=False):
    """Returns (output_scalar, BassKernelResults)."""
    feature = np.ascontiguousarray(np.asarray(feature), dtype=np.float32)
    P = np.asarray(all_prototypes, dtype=np.float32).reshape(C * K, D)
    lbl = int(label)

    nc = _build_bass()
    in_maps = []
    for c in range(N_CORES):
        shard = np.ascontiguousarray(P[c * R : (c + 1) * R])
        in_maps.append({"p": shard, "f": feature})

    res = run_bass_kernel_spmd(
        nc, in_maps, core_ids=list(range(N_CORES)), trace=trace
    )
    outs = res.results

    m = np.stack([o["out_a"][:, NCOLS] for o in outs]).astype(np.float64)
    s = np.stack([o["out_a"][:, NCOLS + 1] for o in outs]).astype(np.float64)
    dsb = [o["out_a"][:, 0:NCOLS] for o in outs]  # each [128, NCOLS] f32

    # all-reduce the scalar denominator (in log space, f64)
    M = float(m.min())
    one = float((s * np.exp(GAMMA * (M - m))).sum())
    log_one = np.log(one) - GAMMA * M

    # numerator: the K rows of the label class live on one shard
    owner, lc = divmod(lbl, CPC)
    dsum = 0.0
    for k in range(K):
        r = lc * K + k
        dsum += float(dsb[owner][r % 128, r // 128])

    prob = K * log_one + GAMMA * dsum
    return np.float32(prob), res


def kernel(feature, label, all_prototypes):
    out, _ = run(feature, label, all_prototypes)
    return out
